# revision 41
# baseline (speedup 1.0000x reference)
"""EmergentVision Trainium2 kernel: conv frontend + 12-step spiking RNN + classifier.

Strategy (8 NeuronCores):
  - Vision frontend: data-parallel over batch (32 images/core), convs as
    tap-packed matmuls, AllGather of bf16 features.
  - Recurrent GEMM: tensor-parallel over the 4096 output dim, batch-stationary
    orientation: out[b,j] accumulates lhsT=spikes.T chunk (bf16, FWL) against
    rhs=w chunk streaming 512 j-columns per matmul. Weights masked on HOST,
    stored bf16 [128,32,512] so every DMA is one contiguous run per partition.
  - LayerNorm: row stats come free via activation/ttr accum_out, one tiny
    [128,4] AllGather of per-core partials, per-partition scale/bias apply on
    the scalar engine.
  - Spikes transposed locally via the PE, AllGathered in bf16.
  - Classifier: per-shard partial matmul + AllReduce (no final spike gather).
"""

import numpy as np

N = 4096
B = 256
NCORES = 8
JS = N // NCORES        # 512 j-shard per core
BS = B // NCORES        # 32 images per core
KC = N // 128           # 32 contraction chunks of the recurrent GEMM
STEPS = 12
LEAK = 0.95
EPS = 1e-5
CONN_THR = 0.01
CDIM = 64 * 7 * 7       # 3136
CKC = 25                # ceil(3137/128); chunk 24 holds cols 3072:3136 + ones row

_PROGRAM_CACHE = {}


def _build_program():
    import concourse.bass as bass
    import concourse.mybir as mybir
    import concourse.tile as tile
    from concourse import bacc
    from concourse.bass import ts, ds
    from concourse.masks import make_identity
    from contextlib import ExitStack

    f32 = mybir.dt.float32
    f32r = mybir.dt.float32r
    bf16 = mybir.dt.bfloat16
    Alu = mybir.AluOpType
    Act = mybir.ActivationFunctionType

    def R(ap):
        return ap.bitcast(f32r)

    def F(ap):
        return ap.bitcast(f32)

    nc = bacc.Bacc("TRN2", target_bir_lowering=False, debug=False,
                   num_devices=NCORES)

    # ---------------- inputs ----------------
    xpad_d = nc.dram_tensor("xpad", [BS, 30, 30], f32, kind="ExternalInput").ap()
    w1t_d = nc.dram_tensor("w1t", [9, 32], f32, kind="ExternalInput").ap()
    w2col_d = nc.dram_tensor("w2col", [3, 96, 64], f32r, kind="ExternalInput").ap()
    bn1_d = nc.dram_tensor("bn1", [4, 32], f32, kind="ExternalInput").ap()
    bn2_d = nc.dram_tensor("bn2", [4, 64], f32, kind="ExternalInput").ap()
    # fc weights [128, 25, 512]: row k=c*128+p holds fc_w.T padded with the
    # bias row at k=3136.
    fcw_d = nc.dram_tensor("fcw", [128, CKC, JS], bf16, kind="ExternalInput").ap()
    # recurrent weights, masked+bf16 on host, k = p*32+c
    wT_d = nc.dram_tensor("wT", [128, KC, JS], bf16, kind="ExternalInput").ap()
    lng_d = nc.dram_tensor("lngr", [1, JS], f32, kind="ExternalInput").ap()
    cb_d = nc.dram_tensor("cbr", [1, JS], f32, kind="ExternalInput").ap()
    thr_d = nc.dram_tensor("thrr", [1, JS], f32, kind="ExternalInput").ap()
    clsw_d = nc.dram_tensor("clsw", [128, 4, 10], bf16, kind="ExternalInput").ap()
    clsb_d = nc.dram_tensor("clsb", [10, 1], f32, kind="ExternalInput").ap()
    # sel4[i, f] = (i % 4 == f): rank-sum selection matrix for the stats AG
    sel4_d = nc.dram_tensor("sel4", [4 * NCORES, 4], f32,
                            kind="ExternalInput").ap()
    out_d = nc.dram_tensor("out", [10, B], f32, kind="ExternalOutput").ap()

    RG = [list(range(NCORES))]

    with tile.TileContext(nc) as tc, ExitStack() as st:
        st.enter_context(nc.allow_low_precision(
            reason="bf16 recurrent weights/spikes are intentional"))
        const = st.enter_context(tc.tile_pool(name="const", bufs=1))
        work = st.enter_context(tc.tile_pool(name="work", bufs=2))
        dram = st.enter_context(tc.tile_pool(name="dram", bufs=2, space="DRAM"))
        psum = st.enter_context(tc.tile_pool(name="psum", bufs=2, space="PSUM"))

        # ---------------- persistent SBUF ----------------
        w_sb = const.tile([128, KC, JS], bf16)        # 4MB masked rec weights
        spT = const.tile([128, KC, B], bf16)          # 2MB gathered spikes.T
        xc = const.tile([128, 2, JS], f32)            # pre-LN GEMM output [b,j]
        q_sb = const.tile([128, 2, JS], f32)          # neuron state min(g,0)
        aq_sb = const.tile([128, 2, JS], f32)         # q*LEAK + cb (pre-hoisted)
        s_sb = const.tile([128, 2, JS], bf16)         # spikes (pre-transpose)
        sp_st = const.tile([128, 4, B], bf16)         # transposed bf16 spikes
        lngrep = const.tile([128, JS], f32)           # ln_g replicated over b
        cbrep = const.tile([128, JS], f32)            # lnb+intr+thr*(LEAK-1) rep
        stat4 = const.tile([128, 4], f32)             # (sx,sxx) x 2 halves
        sel4 = const.tile([4 * NCORES, 4], f32)      # rank-sum selector
        ident = const.tile([128, 128], f32)
        ident_bf = const.tile([128, 128], bf16)
        ones_r = const.tile([1, 128], f32)            # bcast lhsT row
        clsw_sb = const.tile([128, 4, 10], bf16)
        clsb_sb = const.tile([10, 1], f32)
        junk = const.tile([128, JS], bf16)            # ttr spill output

        eps_t = const.tile([128, 1], f32)
        make_identity(nc, ident[:])
        make_identity(nc, ident_bf[:])
        nc.gpsimd.memset(ones_r[:], 1.0)
        nc.gpsimd.memset(eps_t[:], EPS)

        # big persistent loads first so DMA overlaps the conv frontend
        nc.sync.dma_start(w_sb[:], wT_d[:])
        nc.sync.dma_start(clsw_sb[:], clsw_d[:])
        nc.sync.dma_start(clsb_sb[:], clsb_d[:])
        nc.sync.dma_start(sel4[:], sel4_d[:])

        # replicate per-j rows across partitions via K=1 matmuls
        lng_row = work.tile([1, JS], f32, tag="prow", bufs=3, name="lng_row")
        cb_row = work.tile([1, JS], f32, tag="prow", bufs=3, name="cb_row")
        thr_row = work.tile([1, JS], f32, tag="prow", bufs=3, name="thr_row")
        nc.sync.dma_start(lng_row[:], lng_d[:])
        nc.sync.dma_start(cb_row[:], cb_d[:])
        nc.sync.dma_start(thr_row[:], thr_d[:])
        pb = psum.tile([128, JS], f32, tag="gemm", name="pb")
        nc.tensor.matmul(pb[:], ones_r[:], lng_row[:], start=True, stop=True)
        nc.vector.tensor_copy(lngrep[:], pb[:])
        pb2 = psum.tile([128, JS], f32, tag="gemm", name="pb")
        nc.tensor.matmul(pb2[:], ones_r[:], cb_row[:], start=True, stop=True)
        nc.vector.tensor_copy(cbrep[:], pb2[:])
        pb3 = psum.tile([128, JS], f32, tag="gemm", name="pb")
        nc.tensor.matmul(pb3[:], ones_r[:], thr_row[:], start=True, stop=True)
        # q init: potential=0 -> q = -thr;  aq = q*LEAK + cb
        for h in range(2):
            nc.scalar.mul(q_sb[:, h, :], pb3[:], -1.0)
            nc.scalar.mul(aq_sb[:, h, :], pb3[:], -LEAK)
            nc.vector.tensor_tensor(aq_sb[:, h, :], aq_sb[:, h, :], cbrep[:],
                                    Alu.add)

        # ---------------- bn params ----------------
        def bn_prep(bn_d, nchan, nrep):
            p = nchan * nrep
            raw = const.tile([p, 4], f32, name=f"bnraw{nchan}")
            for rep in range(nrep):
                nc.sync.dma_start(raw[rep * nchan:(rep + 1) * nchan, :],
                                  bn_d.rearrange("f c -> c f"))
            s_t = const.tile([p, 1], f32, name=f"bns{nchan}")
            t_t = const.tile([p, 1], f32, name=f"bnt{nchan}")
            tmp = work.tile([p, 1], f32, tag="bntmp")
            # s = g / sqrt(v + eps); t = b - m*s
            nc.vector.tensor_scalar_add(tmp[:], raw[:, 3:4], EPS)
            nc.scalar.activation(tmp[:], tmp[:], Act.Sqrt)
            nc.vector.reciprocal(s_t[:], tmp[:])
            nc.vector.tensor_tensor(s_t[:], s_t[:], raw[:, 0:1], Alu.mult)
            nc.vector.tensor_tensor(tmp[:], raw[:, 2:3], s_t[:], Alu.mult)
            nc.vector.tensor_tensor(t_t[:], raw[:, 1:2], tmp[:], Alu.subtract)
            return s_t, t_t

        bn1s, bn1t = bn_prep(bn1_d, 32, 4)
        bn2s, bn2t = bn_prep(bn2_d, 64, 1)

        # ---------------- conv frontend ----------------
        h_loc_d = dram.tile([BS, CDIM], bf16, bufs=1)

        with tc.tile_pool(name="convA", bufs=1) as convA:
            x2pad4 = convA.tile([128, 8, 16, 16], f32r)
            nc.gpsimd.memset(F(x2pad4[:]), 0.0)
            h_sb = convA.tile([64, BS, 49], bf16)

            with tc.tile_pool(name="conv1", bufs=1) as conv1p:
                rhs9 = conv1p.tile([128, 6272], f32)
                act1 = rhs9  # chunk columns are dead after their matmuls
                w1rep = conv1p.tile([128, 32], f32)
                for bq in range(4):
                    nc.sync.dma_start(w1rep[32 * bq:32 * bq + 9, :], w1t_d[:])
                    for t in range(9):
                        dy, dx = t // 3, t % 3
                        dst = rhs9[32 * bq + t:32 * bq + t + 1, :].rearrange(
                            "p (b i j) -> p b i j", b=8, i=28, j=28)
                        nc.sync.dma_start(
                            dst, xpad_d[bq * 8:(bq + 1) * 8,
                                        dy:dy + 28, dx:dx + 28][None])
                # conv1 matmuls: 4 col/row-tiled strips, 13 chunks each
                offs = [(i * 512, 512) for i in range(12)] + [(6144, 128)]
                for (off, cw) in offs:
                    ps1 = psum.tile([128, 512], f32, tag="gemm", name="ps1")
                    for bq in range(4):
                        nc.tensor.matmul(
                            ps1[32 * bq:32 * bq + 32, :cw],
                            w1rep[32 * bq:32 * bq + 9, :],
                            rhs9[32 * bq:32 * bq + 9, ds(off, cw)],
                            start=True, stop=True,
                            tile_position=(32 * bq, 32 * bq))
                    nc.scalar.activation(act1[:, ds(off, cw)], ps1[:, :cw],
                                         Act.Relu, bias=bn1t[:], scale=bn1s[:])
                # maxpool 2x2 -> write into padded x2pad4 interior
                av = act1.rearrange("p (b i2 iw j2 jw) -> p b i2 iw j2 jw",
                                    b=8, i2=14, iw=2, j2=14, jw=2)
                m1 = conv1p.tile([128, 8, 14, 14], f32)
                nc.any.tensor_tensor(m1[:], av[:, :, :, 0, :, 0],
                                     av[:, :, :, 0, :, 1], Alu.max)
                nc.any.tensor_tensor(x2pad4[:, :, 1:15, 1:15],
                                     av[:, :, :, 1, :, 0],
                                     av[:, :, :, 1, :, 1], Alu.max)
                nc.any.tensor_tensor(x2pad4[:, :, 1:15, 1:15],
                                     F(x2pad4[:, :, 1:15, 1:15]), m1[:],
                                     Alu.max)

            with tc.tile_pool(name="conv2", bufs=1) as conv2p:
                w2_sb = conv2p.tile([128, 3, 64], f32r)
                nc.sync.dma_start(w2_sb[0:96, :, :],
                                  w2col_d.rearrange("d p o -> p d o"))
                hv = h_sb.rearrange("p b (i j) -> p b i j", i=7, j=7)
                for bh in range(2):  # two halves of 16 images
                    x2col = conv2p.tile([128, 16, 16, 16], f32r, tag="x2col")
                    act2 = conv2p.tile([64, 16 * 196], f32, tag="act2")
                    for tx in range(3):
                        for bq in range(2 * bh, 2 * bh + 2):
                            bo = (bq - 2 * bh) * 8
                            nc.sync.dma_start(
                                x2col[32 * tx:32 * tx + 32,
                                      bo:bo + 8, :, 0:16 - tx],
                                x2pad4[32 * bq:32 * bq + 32, :, :, tx:16])
                    for ch in range(8):
                        ps2 = psum.tile([64, 392], f32, tag="c2", bufs=1, name="ps2")
                        for dy in range(3):
                            nc.tensor.matmul(
                                ps2[:],
                                R(w2_sb[0:96, dy, :]),
                                R(x2col[0:96, 2 * ch:2 * ch + 2,
                                        dy:dy + 14, 0:14]),
                                start=(dy == 0), stop=(dy == 2))
                        nc.scalar.activation(
                            act2[:, ds(ch * 392, 392)], ps2[:], Act.Relu,
                            bias=bn2t[:], scale=bn2s[:])
                    # maxpool 2x2 -> h [64, 16, 7, 7] for this half
                    av2 = act2.rearrange(
                        "p (b i2 iw j2 jw) -> p b i2 iw j2 jw",
                        b=16, i2=7, iw=2, j2=7, jw=2)
                    n1 = conv2p.tile([64, 16, 7, 7], f32, tag="n1")
                    hvh = hv[:, bh * 16:(bh + 1) * 16, :, :]
                    nc.any.tensor_tensor(n1[:], av2[:, :, :, 0, :, 0],
                                         av2[:, :, :, 0, :, 1], Alu.max)
                    nc.any.tensor_tensor(hvh, av2[:, :, :, 1, :, 0],
                                         av2[:, :, :, 1, :, 1], Alu.max)
                    nc.any.tensor_tensor(hvh, hvh, n1[:], Alu.max)

            # h -> DRAM as [b, c] with c = oc*49 + ij
            nc.sync.dma_start(
                h_loc_d.rearrange("b (oc ij) -> oc b ij", oc=64), h_sb[:])

        # AllGather h across cores -> [256, 3136] bf16
        h_all_d = dram.tile([B, CDIM], bf16, bufs=1, addr_space="Shared")
        nc.gpsimd.collective_compute(
            "AllGather", Alu.bypass, replica_groups=RG,
            ins=[h_loc_d[:].opt()], outs=[h_all_d[:].opt()])

        # ---------------- transpose h, fc GEMM (batch-stationary) ----------
        px = [psum.tile([128, JS], f32, tag="gemm", name=f"px{h}")
              for h in range(2)]
        with tc.tile_pool(name="fcp", bufs=1) as fcp:
            fcw_sb = fcp.tile([128, CKC, JS], bf16)
            nc.sync.dma_start(fcw_sb[:], fcw_d[:])
            hT = fcp.tile([128, CKC, B], bf16)
            nc.gpsimd.memset(hT[64:, CKC - 1, :], 0.0)
            nc.gpsimd.memset(hT[64:65, CKC - 1, :], 1.0)  # fc bias row
            for bt in range(2):
                hall = fcp.tile([128, CDIM], bf16, tag="hall", bufs=1)
                nc.sync.dma_start(hall[:], h_all_d[bt * 128:(bt + 1) * 128, :])
                for cc in range(CKC):
                    cw = 128 if cc < CKC - 1 else 64
                    pt = psum.tile([128, 128], bf16, tag="tr", bufs=3,
                                   name="ptrh")
                    nc.tensor.transpose(pt[:cw, :], hall[:, ds(cc * 128, cw)],
                                        ident_bf[:])
                    nc.any.tensor_copy(hT[:cw, cc, ts(bt, 128)], pt[:cw, :])
            for h in range(2):
                for cc in range(CKC):
                    nc.tensor.matmul(px[h][:], hT[:, cc, ts(h, 128)],
                                     fcw_sb[:, cc, :],
                                     start=(cc == 0), stop=(cc == CKC - 1))

        # ---------------- recurrent steps ----------------
        def step_body(step, pxs):
            # --- stats: row sums via accum_out, cross-core gather, finalize
            # stat4 cols: [sx_h0, sx_h1, sxx_h0, sxx_h1]
            for h in range(2):
                nc.scalar.activation(xc[:, h, :], pxs[h][:], Act.Identity,
                                     accum_out=stat4[:, h:h + 1])
            for h in range(2):
                nc.scalar.activation(junk[:], xc[:, h, :], Act.Square,
                                     accum_out=stat4[:, 2 + h:3 + h])
            # ship stats TRANSPOSED [4, 128] so every DMA is a handful of
            # descriptors (the queues process ~25M desc/s serially).
            pst = psum.tile([4, 128], f32, tag="cls", bufs=1, name="pst")
            nc.tensor.transpose(pst[:], stat4[:], ident[:])
            stT = work.tile([4, 128], f32, tag="stT", name="stT")
            nc.vector.tensor_copy(stT[:], pst[:])
            st_in = dram.tile([4, 128], f32, tag="stin", name="st_in")
            st_out = dram.tile([4 * NCORES, 128], f32, tag="stout",
                               addr_space="Shared", name="st_out")
            nc.sync.dma_start(st_in[:], stT[:])
            nc.gpsimd.collective_compute(
                "AllGather", Alu.bypass, replica_groups=RG,
                ins=[st_in[:].opt()], outs=[st_out[:].opt()])
            stg = work.tile([4 * NCORES, 128], f32, tag="stg", name="stg")
            nc.sync.dma_start(stg[:], st_out[:])
            # rank-sum via sel4, then transpose back to per-partition [128, 4]
            psr = psum.tile([4, 128], f32, tag="cls", bufs=1, name="psr")
            nc.tensor.matmul(psr[:], sel4[:], stg[:], start=True, stop=True)
            stf = work.tile([4, 128], f32, tag="stT", name="stf")
            nc.vector.tensor_copy(stf[:], psr[:])
            pfin = psum.tile([128, 4], f32, tag="cls", bufs=1, name="pfin")
            nc.tensor.transpose(pfin[:], stf[:], ident[0:4, 0:4])
            tot = work.tile([128, 4], f32, tag="invnc", bufs=3, name="tot")
            nc.vector.tensor_copy(tot[:], pfin[:])
            # cols of tot: [sx_h0, sx_h1, sxx_h0, sxx_h1]
            totv = tot.rearrange("p (s h) -> p s h", s=2)
            negmu = work.tile([128, 2], f32, tag="fin", bufs=8, name="negmu")
            msq = work.tile([128, 2], f32, tag="fin", bufs=8, name="msq")
            var = work.tile([128, 2], f32, tag="fin", bufs=8, name="var")
            inv2 = work.tile([128, 2], f32, tag="fin", bufs=8, name="inv2")
            ncmu = work.tile([128, 2], f32, tag="fin", bufs=8, name="ncmu")
            nc.vector.tensor_scalar_mul(negmu[:], totv[:, 0, :], -1.0 / N)
            nc.vector.tensor_scalar_mul(var[:], totv[:, 1, :], 1.0 / N)
            nc.vector.tensor_tensor(msq[:], negmu[:], negmu[:], Alu.mult)
            nc.vector.tensor_tensor(var[:], var[:], msq[:], Alu.subtract)
            nc.scalar.activation(var[:], var[:], Act.Sqrt, bias=eps_t[:])
            nc.vector.reciprocal(inv2[:], var[:])
            nc.vector.tensor_tensor(ncmu[:], negmu[:], inv2[:], Alu.mult)

            # --- neuron update per batch-half [128, 512]
            # critical path: t1 -> m1 -> g -> silu -> relu -> transpose.
            # q / aq-for-next-step are off-path (scheduler hoists them).
            for h in range(2):
                # t1 on the DVE so the scalar engine goes straight to Silu —
                # its act-table load hides behind this chain.
                t1 = work.tile([128, JS], f32, tag="t1", name=f"t1{h}")
                nc.vector.tensor_scalar(t1[:], xc[:, h, :],
                                        inv2[:, h:h + 1], ncmu[:, h:h + 1],
                                        Alu.mult, Alu.add)
                m1 = work.tile([128, JS], f32, tag="m1", name=f"m1{h}")
                eng = nc.vector if h == 0 else nc.gpsimd
                eng.tensor_tensor(m1[:], t1[:], lngrep[:], Alu.mult)
                g = work.tile([128, JS], f32, tag="g", name=f"g{h}")
                nc.vector.tensor_tensor(g[:], m1[:], aq_sb[:, h, :], Alu.add)
                sv = work.tile([128, JS], f32, tag="sv", name=f"sv{h}")
                nc.scalar.activation(sv[:], g[:], Act.Silu)
                # spikes = silu(g) * (g>0) == relu(silu(g))
                nc.scalar.activation(s_sb[:, h, :], sv[:], Act.Relu)
                nc.vector.tensor_scalar_min(q_sb[:, h, :], g[:], 0.0)
                nc.vector.scalar_tensor_tensor(aq_sb[:, h, :], q_sb[:, h, :],
                                               LEAK, cbrep[:], Alu.mult,
                                               Alu.add)
        def transpose_spikes(step):
            # transpose spikes to [j, b] bf16, DMA per block pair
            sp_in = dram.tile([JS, B], bf16, tag="spin", name="sp_in")
            for t in range(4):
                for h in range(2):
                    pt = psum.tile([128, 128], bf16, tag="tr", bufs=3,
                                   name="ptr")
                    nc.tensor.transpose(pt[:], s_sb[:, h, ds(t * 128, 128)],
                                        ident_bf[:])
                    if (t * 2 + h) % 2 == 0:
                        nc.scalar.copy(sp_st[:, t, ts(h, 128)], pt[:])
                    else:
                        nc.vector.tensor_copy(sp_st[:, t, ts(h, 128)], pt[:])
                nc.sync.dma_start(sp_in[ds(t * 128, 64), :],
                                  sp_st[0:64, t, :])
                nc.sync.dma_start(sp_in[ds(t * 128 + 64, 64), :],
                                  sp_st[64:128, t, :])
            return sp_in

        def spikes_ag(sp_in):
            ag_out = dram.tile([N, B], bf16, tag="agout", addr_space="Shared",
                               name="ag_out")
            nc.gpsimd.collective_compute(
                "AllGather", Alu.bypass, replica_groups=RG,
                ins=[sp_in[:].opt()], outs=[ag_out[:].opt()])
            return ag_out

        def recurrent_gemm(ag_out):
            # ag row k = r*512 + jl; spT[p, c] holds k(p,c) =
            # (p//16)*512 + (c//16)*256 + (p%16)*16 + (c%16), matching the
            # host-side weight permutation. Progressive loads let the first
            # matmuls start before the whole 512KB lands.
            for s_ in range(2):
                for r in range(NCORES):
                    blk = ag_out[ds(r * 512 + s_ * 256, 256), :].rearrange(
                        "(ph c2) b -> ph c2 b", ph=16)
                    nc.sync.dma_start(
                        spT[ds(r * 16, 16), ds(s_ * 16, 16), :], blk)
            pxs = [psum.tile([128, JS], f32, tag="gemm", name=f"px{h}")
                   for h in range(2)]
            for h in range(2):
                for c in range(KC):
                    nc.tensor.matmul(pxs[h][:], spT[:, c, ds(h * 128, 128)],
                                     w_sb[:, c, :],
                                     start=(c == 0), stop=(c == KC - 1))
            return pxs

        for step in range(STEPS):
            step_body(step, px)
            sp_ins = transpose_spikes(step)
            if step < STEPS - 1:
                ags = spikes_ag(sp_ins)
                px = recurrent_gemm(ags)

        # ---------------- classifier: partial + AllReduce ----------------
        ps_cls = psum.tile([10, B], f32, tag="cls", bufs=1, name="ps_cls")
        for t in range(4):
            nc.tensor.matmul(ps_cls[:], clsw_sb[:, t, :], sp_st[:, t, :],
                             start=(t == 0), stop=(t == 3))
        cls_loc = work.tile([10, B], f32, tag="clsl", name="cls_loc")
        nc.scalar.copy(cls_loc[:], ps_cls[:])
        cls_in = dram.tile([10, B], f32, bufs=1)
        cls_out = dram.tile([10 * NCORES, B], f32, bufs=1, addr_space="Shared")
        nc.sync.dma_start(cls_in[:], cls_loc[:])
        nc.gpsimd.collective_compute(
            "AllGather", Alu.bypass, replica_groups=RG,
            ins=[cls_in[:].opt()], outs=[cls_out[:].opt()])
        cls_sb = work.tile([10, NCORES, B], f32, tag="clsg", name="cls_sb")
        nc.sync.dma_start(cls_sb[:],
                          cls_out.rearrange("(r p) b -> p r b", p=10))
        acc = work.tile([10, B], f32, tag="clsl", name="acc")
        nc.vector.tensor_tensor(acc[:], cls_sb[:, 0, :], cls_sb[:, 1, :],
                                Alu.add)
        for r in range(2, NCORES):
            nc.vector.tensor_tensor(acc[:], acc[:], cls_sb[:, r, :], Alu.add)
        out_sb = work.tile([10, B], f32, tag="clsl", name="out_sb")
        nc.scalar.activation(out_sb[:], acc[:], Act.Identity,
                             bias=clsb_sb[:])
        nc.sync.dma_start(out_d[:], out_sb[:])

    nc.compile()
    return nc


def _bf16(a):
    """Round fp32 -> bf16 (round-to-nearest-even), keep bf16 dtype via ml_dtypes."""
    import ml_dtypes
    return np.ascontiguousarray(a, np.float32).astype(ml_dtypes.bfloat16)


def _host_prep(inputs):
    """Shard + lay out the full inputs for the 8 cores."""
    x = np.asarray(inputs["x"], np.float32)
    xpad = np.zeros((B, 30, 30), np.float32)
    xpad[:, 1:29, 1:29] = x[:, 0]
    w1t = np.ascontiguousarray(
        np.asarray(inputs["conv1_w"], np.float32).reshape(32, 9).T)

    def _round_f32r(a):
        b = np.ascontiguousarray(a, np.float32).view(np.uint32).astype(np.uint64)
        lsb = (b >> 12) & 1
        out = ((b + 0x7FF + lsb) & 0xFFFFF000).astype(np.uint32)
        return out.view(np.float32)

    w2col = _round_f32r(np.ascontiguousarray(
        np.asarray(inputs["conv2_w"], np.float32).transpose(2, 3, 1, 0)
        .reshape(3, 96, 64)))
    bn1 = np.stack([inputs["bn1_g"], inputs["bn1_b"],
                    inputs["bn1_m"], inputs["bn1_v"]]).astype(np.float32)
    bn2 = np.stack([inputs["bn2_g"], inputs["bn2_b"],
                    inputs["bn2_m"], inputs["bn2_v"]]).astype(np.float32)
    fc_w = np.asarray(inputs["fc_w"], np.float32)
    fc_b = np.asarray(inputs["fc_b"], np.float32)
    rec_w = np.asarray(inputs["rec_w"], np.float32)
    eff_w = rec_w * (np.abs(rec_w) > CONN_THR)
    cls_w = np.asarray(inputs["cls_w"], np.float32)
    clsb = np.ascontiguousarray(
        np.asarray(inputs["cls_b"], np.float32).reshape(10, 1))
    lng = np.asarray(inputs["ln_g"], np.float32)
    lnb = np.asarray(inputs["ln_b"], np.float32)
    thr = np.asarray(inputs["threshold"], np.float32)
    intr = np.asarray(inputs["intrinsic"], np.float32)
    cb = lnb + intr + thr * (LEAK - 1.0)

    in_maps = []
    for r in range(NCORES):
        js = slice(r * JS, (r + 1) * JS)
        # fc weights: [3137 padded to 3200, 512] -> [128, 25, 512], k=c*128+p
        fcp = np.zeros((CKC * 128, JS), np.float32)
        fcp[0:CDIM] = fc_w[js].T
        fcp[CDIM] = fc_b[js]
        fcw = _bf16(fcp.reshape(CKC, 128, JS).transpose(1, 0, 2))
        # recurrent: [4096, 512] -> [128, 32, 512] with
        # k(p, c) = (p//16)*512 + (c//16)*256 + (p%16)*16 + (c%16)
        # (matches the split-AG output layout: p=(r,ph), c=(s,c2))
        wT = _bf16(np.ascontiguousarray(
            eff_w[js].T.reshape(8, 2, 16, 16, JS)
            .transpose(0, 2, 1, 3, 4).reshape(128, KC, JS)))
        clswT = _bf16(np.ascontiguousarray(cls_w[:, js].T)
                      .reshape(4, 128, 10).transpose(1, 0, 2))
        in_maps.append(dict(
            xpad=np.ascontiguousarray(xpad[r * BS:(r + 1) * BS]),
            w1t=w1t, w2col=w2col, bn1=bn1, bn2=bn2,
            fcw=np.ascontiguousarray(fcw),
            wT=np.ascontiguousarray(wT),
            lngr=np.ascontiguousarray(lng[js].reshape(1, JS)),
            cbr=np.ascontiguousarray(cb[js].reshape(1, JS)),
            thrr=np.ascontiguousarray(thr[js].reshape(1, JS)),
            clsw=np.ascontiguousarray(clswT), clsb=clsb,
            sel4=np.ascontiguousarray(
                np.tile(np.eye(4, dtype=np.float32), (NCORES, 1))),
        ))
    return in_maps


def kernel(**inputs) -> np.ndarray:
    from concourse import bass_utils

    if "nc" not in _PROGRAM_CACHE:
        _PROGRAM_CACHE["nc"] = _build_program()
    nc = _PROGRAM_CACHE["nc"]

    in_maps = _host_prep(inputs)
    res = bass_utils.run_bass_kernel_spmd(
        nc, in_maps, core_ids=list(range(NCORES)))
    _PROGRAM_CACHE["last_results"] = res
    out = res.results[0]["out"]
    return np.ascontiguousarray(out.T.astype(np.float32))


# revision 42
# speedup vs baseline: 1.1373x; 1.1373x over previous
"""EmergentVision Trainium2 kernel: conv frontend + 12-step spiking RNN + classifier.

Strategy (8 NeuronCores):
  - Vision frontend: data-parallel over batch (32 images/core), convs as
    tap-packed matmuls, AllGather of bf16 features.
  - Recurrent GEMM: tensor-parallel over the 4096 output dim, batch-stationary
    orientation: out[b,j] accumulates lhsT=spikes.T chunk (bf16, FWL) against
    rhs=w chunk streaming 512 j-columns per matmul. Weights masked on HOST,
    stored bf16 [128,32,512] so every DMA is one contiguous run per partition.
  - LayerNorm: row stats come free via activation/ttr accum_out, one tiny
    [128,4] AllGather of per-core partials, per-partition scale/bias apply on
    the scalar engine.
  - Spikes transposed locally via the PE, AllGathered in bf16.
  - Classifier: per-shard partial matmul + AllReduce (no final spike gather).
"""

import numpy as np

N = 4096
B = 256
NCORES = 8
JS = N // NCORES        # 512 j-shard per core
BS = B // NCORES        # 32 images per core
KC = N // 128           # 32 contraction chunks of the recurrent GEMM
STEPS = 12
LEAK = 0.95
EPS = 1e-5
CONN_THR = 0.01
CDIM = 64 * 7 * 7       # 3136
CKC = 25                # ceil(3137/128); chunk 24 holds cols 3072:3136 + ones row

_PROGRAM_CACHE = {}


def _build_program():
    import concourse.bass as bass
    import concourse.mybir as mybir
    import concourse.tile as tile
    from concourse import bacc
    from concourse.bass import ts, ds
    from concourse.masks import make_identity
    from contextlib import ExitStack

    f32 = mybir.dt.float32
    f32r = mybir.dt.float32r
    bf16 = mybir.dt.bfloat16
    Alu = mybir.AluOpType
    Act = mybir.ActivationFunctionType

    def R(ap):
        return ap.bitcast(f32r)

    def F(ap):
        return ap.bitcast(f32)

    nc = bacc.Bacc("TRN2", target_bir_lowering=False, debug=False,
                   num_devices=NCORES)

    # ---------------- inputs ----------------
    xpad_d = nc.dram_tensor("xpad", [BS, 30, 30], f32, kind="ExternalInput").ap()
    w1t_d = nc.dram_tensor("w1t", [9, 32], f32, kind="ExternalInput").ap()
    w2col_d = nc.dram_tensor("w2col", [3, 96, 64], f32r, kind="ExternalInput").ap()
    bn1_d = nc.dram_tensor("bn1", [4, 32], f32, kind="ExternalInput").ap()
    bn2_d = nc.dram_tensor("bn2", [4, 64], f32, kind="ExternalInput").ap()
    # fc weights [128, 25, 512]: row k=c*128+p holds fc_w.T padded with the
    # bias row at k=3136.
    fcw_d = nc.dram_tensor("fcw", [128, CKC, JS], bf16, kind="ExternalInput").ap()
    # recurrent weights, masked+bf16 on host, k = p*32+c
    wT_d = nc.dram_tensor("wT", [128, KC, JS], bf16, kind="ExternalInput").ap()
    lng_d = nc.dram_tensor("lngr", [1, JS], f32, kind="ExternalInput").ap()
    cb_d = nc.dram_tensor("cbr", [1, JS], f32, kind="ExternalInput").ap()
    thr_d = nc.dram_tensor("thrr", [1, JS], f32, kind="ExternalInput").ap()
    clsw_d = nc.dram_tensor("clsw", [128, 4, 10], bf16, kind="ExternalInput").ap()
    clsb_d = nc.dram_tensor("clsb", [10, 1], f32, kind="ExternalInput").ap()
    out_d = nc.dram_tensor("out", [10, B], f32, kind="ExternalOutput").ap()

    RG = [list(range(NCORES))]

    with tile.TileContext(nc) as tc, ExitStack() as st:
        st.enter_context(nc.allow_low_precision(
            reason="bf16 recurrent weights/spikes are intentional"))
        const = st.enter_context(tc.tile_pool(name="const", bufs=1))
        work = st.enter_context(tc.tile_pool(name="work", bufs=2))
        dram = st.enter_context(tc.tile_pool(name="dram", bufs=2, space="DRAM"))
        psum = st.enter_context(tc.tile_pool(name="psum", bufs=2, space="PSUM"))

        # ---------------- persistent SBUF ----------------
        w_sb = const.tile([128, KC, JS], bf16)        # 4MB masked rec weights
        spT = const.tile([128, KC, B], bf16)          # 2MB gathered spikes.T
        xc = const.tile([128, 2, JS], f32)            # pre-LN GEMM output [b,j]
        q_sb = const.tile([128, 2, JS], f32)          # neuron state min(g,0)
        aq_sb = const.tile([128, 2, JS], f32)         # q*LEAK + cb (pre-hoisted)
        s_sb = const.tile([128, 2, JS], bf16)         # spikes (pre-transpose)
        sp_st = const.tile([128, 4, B], bf16)         # transposed bf16 spikes
        lngrep = const.tile([128, JS], f32)           # ln_g replicated over b
        cbrep = const.tile([128, JS], f32)            # lnb+intr+thr*(LEAK-1) rep
        stat4 = const.tile([128, 4], f32)             # (sx,sxx) x 2 halves
        ident = const.tile([128, 128], f32)
        ident_bf = const.tile([128, 128], bf16)
        ones_r = const.tile([1, 128], f32)            # bcast lhsT row
        clsw_sb = const.tile([128, 4, 10], bf16)
        clsb_sb = const.tile([10, 1], f32)
        junk = const.tile([128, JS], bf16)            # ttr spill output

        eps_t = const.tile([128, 1], f32)
        make_identity(nc, ident[:])
        make_identity(nc, ident_bf[:])
        nc.gpsimd.memset(ones_r[:], 1.0)
        nc.gpsimd.memset(eps_t[:], EPS)

        # big persistent loads first so DMA overlaps the conv frontend
        nc.sync.dma_start(w_sb[:], wT_d[:])
        nc.sync.dma_start(clsw_sb[:], clsw_d[:])
        nc.sync.dma_start(clsb_sb[:], clsb_d[:])

        # replicate per-j rows across partitions via K=1 matmuls
        lng_row = work.tile([1, JS], f32, tag="prow", bufs=3, name="lng_row")
        cb_row = work.tile([1, JS], f32, tag="prow", bufs=3, name="cb_row")
        thr_row = work.tile([1, JS], f32, tag="prow", bufs=3, name="thr_row")
        nc.sync.dma_start(lng_row[:], lng_d[:])
        nc.sync.dma_start(cb_row[:], cb_d[:])
        nc.sync.dma_start(thr_row[:], thr_d[:])
        pb = psum.tile([128, JS], f32, tag="gemm", name="pb")
        nc.tensor.matmul(pb[:], ones_r[:], lng_row[:], start=True, stop=True)
        nc.vector.tensor_copy(lngrep[:], pb[:])
        pb2 = psum.tile([128, JS], f32, tag="gemm", name="pb")
        nc.tensor.matmul(pb2[:], ones_r[:], cb_row[:], start=True, stop=True)
        nc.vector.tensor_copy(cbrep[:], pb2[:])
        pb3 = psum.tile([128, JS], f32, tag="gemm", name="pb")
        nc.tensor.matmul(pb3[:], ones_r[:], thr_row[:], start=True, stop=True)
        # q init: potential=0 -> q = -thr;  aq = q*LEAK + cb
        for h in range(2):
            nc.scalar.mul(q_sb[:, h, :], pb3[:], -1.0)
            nc.scalar.mul(aq_sb[:, h, :], pb3[:], -LEAK)
            nc.vector.tensor_tensor(aq_sb[:, h, :], aq_sb[:, h, :], cbrep[:],
                                    Alu.add)

        # ---------------- bn params ----------------
        def bn_prep(bn_d, nchan, nrep):
            p = nchan * nrep
            raw = const.tile([p, 4], f32, name=f"bnraw{nchan}")
            for rep in range(nrep):
                nc.sync.dma_start(raw[rep * nchan:(rep + 1) * nchan, :],
                                  bn_d.rearrange("f c -> c f"))
            s_t = const.tile([p, 1], f32, name=f"bns{nchan}")
            t_t = const.tile([p, 1], f32, name=f"bnt{nchan}")
            tmp = work.tile([p, 1], f32, tag="bntmp")
            # s = g / sqrt(v + eps); t = b - m*s
            nc.vector.tensor_scalar_add(tmp[:], raw[:, 3:4], EPS)
            nc.scalar.activation(tmp[:], tmp[:], Act.Sqrt)
            nc.vector.reciprocal(s_t[:], tmp[:])
            nc.vector.tensor_tensor(s_t[:], s_t[:], raw[:, 0:1], Alu.mult)
            nc.vector.tensor_tensor(tmp[:], raw[:, 2:3], s_t[:], Alu.mult)
            nc.vector.tensor_tensor(t_t[:], raw[:, 1:2], tmp[:], Alu.subtract)
            return s_t, t_t

        bn1s, bn1t = bn_prep(bn1_d, 32, 4)
        bn2s, bn2t = bn_prep(bn2_d, 64, 1)

        # ---------------- conv frontend ----------------
        h_loc_d = dram.tile([BS, CDIM], bf16, bufs=1)

        with tc.tile_pool(name="convA", bufs=1) as convA:
            x2pad4 = convA.tile([128, 8, 16, 16], f32r)
            nc.gpsimd.memset(F(x2pad4[:]), 0.0)
            h_sb = convA.tile([64, BS, 49], bf16)

            with tc.tile_pool(name="conv1", bufs=1) as conv1p:
                rhs9 = conv1p.tile([128, 6272], f32)
                act1 = rhs9  # chunk columns are dead after their matmuls
                w1rep = conv1p.tile([128, 32], f32)
                for bq in range(4):
                    nc.sync.dma_start(w1rep[32 * bq:32 * bq + 9, :], w1t_d[:])
                    for t in range(9):
                        dy, dx = t // 3, t % 3
                        dst = rhs9[32 * bq + t:32 * bq + t + 1, :].rearrange(
                            "p (b i j) -> p b i j", b=8, i=28, j=28)
                        nc.sync.dma_start(
                            dst, xpad_d[bq * 8:(bq + 1) * 8,
                                        dy:dy + 28, dx:dx + 28][None])
                # conv1 matmuls: 4 col/row-tiled strips, 13 chunks each
                offs = [(i * 512, 512) for i in range(12)] + [(6144, 128)]
                for (off, cw) in offs:
                    ps1 = psum.tile([128, 512], f32, tag="gemm", name="ps1")
                    for bq in range(4):
                        nc.tensor.matmul(
                            ps1[32 * bq:32 * bq + 32, :cw],
                            w1rep[32 * bq:32 * bq + 9, :],
                            rhs9[32 * bq:32 * bq + 9, ds(off, cw)],
                            start=True, stop=True,
                            tile_position=(32 * bq, 32 * bq))
                    nc.scalar.activation(act1[:, ds(off, cw)], ps1[:, :cw],
                                         Act.Relu, bias=bn1t[:], scale=bn1s[:])
                # maxpool 2x2 -> write into padded x2pad4 interior
                av = act1.rearrange("p (b i2 iw j2 jw) -> p b i2 iw j2 jw",
                                    b=8, i2=14, iw=2, j2=14, jw=2)
                m1 = conv1p.tile([128, 8, 14, 14], f32)
                nc.any.tensor_tensor(m1[:], av[:, :, :, 0, :, 0],
                                     av[:, :, :, 0, :, 1], Alu.max)
                nc.any.tensor_tensor(x2pad4[:, :, 1:15, 1:15],
                                     av[:, :, :, 1, :, 0],
                                     av[:, :, :, 1, :, 1], Alu.max)
                nc.any.tensor_tensor(x2pad4[:, :, 1:15, 1:15],
                                     F(x2pad4[:, :, 1:15, 1:15]), m1[:],
                                     Alu.max)

            with tc.tile_pool(name="conv2", bufs=1) as conv2p:
                w2_sb = conv2p.tile([128, 3, 64], f32r)
                nc.sync.dma_start(w2_sb[0:96, :, :],
                                  w2col_d.rearrange("d p o -> p d o"))
                hv = h_sb.rearrange("p b (i j) -> p b i j", i=7, j=7)
                for bh in range(2):  # two halves of 16 images
                    x2col = conv2p.tile([128, 16, 16, 16], f32r, tag="x2col")
                    act2 = conv2p.tile([64, 16 * 196], f32, tag="act2")
                    for tx in range(3):
                        for bq in range(2 * bh, 2 * bh + 2):
                            bo = (bq - 2 * bh) * 8
                            nc.sync.dma_start(
                                x2col[32 * tx:32 * tx + 32,
                                      bo:bo + 8, :, 0:16 - tx],
                                x2pad4[32 * bq:32 * bq + 32, :, :, tx:16])
                    for ch in range(8):
                        ps2 = psum.tile([64, 392], f32, tag="c2", bufs=1, name="ps2")
                        for dy in range(3):
                            nc.tensor.matmul(
                                ps2[:],
                                R(w2_sb[0:96, dy, :]),
                                R(x2col[0:96, 2 * ch:2 * ch + 2,
                                        dy:dy + 14, 0:14]),
                                start=(dy == 0), stop=(dy == 2))
                        nc.scalar.activation(
                            act2[:, ds(ch * 392, 392)], ps2[:], Act.Relu,
                            bias=bn2t[:], scale=bn2s[:])
                    # maxpool 2x2 -> h [64, 16, 7, 7] for this half
                    av2 = act2.rearrange(
                        "p (b i2 iw j2 jw) -> p b i2 iw j2 jw",
                        b=16, i2=7, iw=2, j2=7, jw=2)
                    n1 = conv2p.tile([64, 16, 7, 7], f32, tag="n1")
                    hvh = hv[:, bh * 16:(bh + 1) * 16, :, :]
                    nc.any.tensor_tensor(n1[:], av2[:, :, :, 0, :, 0],
                                         av2[:, :, :, 0, :, 1], Alu.max)
                    nc.any.tensor_tensor(hvh, av2[:, :, :, 1, :, 0],
                                         av2[:, :, :, 1, :, 1], Alu.max)
                    nc.any.tensor_tensor(hvh, hvh, n1[:], Alu.max)

            # h -> DRAM as [b, c] with c = oc*49 + ij
            nc.sync.dma_start(
                h_loc_d.rearrange("b (oc ij) -> oc b ij", oc=64), h_sb[:])

        # AllGather h across cores -> [256, 3136] bf16
        h_all_d = dram.tile([B, CDIM], bf16, bufs=1, addr_space="Shared")
        nc.gpsimd.collective_compute(
            "AllGather", Alu.bypass, replica_groups=RG,
            ins=[h_loc_d[:].opt()], outs=[h_all_d[:].opt()])

        # ---------------- transpose h, fc GEMM (batch-stationary) ----------
        px = [psum.tile([128, JS], f32, tag="gemm", name=f"px{h}")
              for h in range(2)]
        with tc.tile_pool(name="fcp", bufs=1) as fcp:
            fcw_sb = fcp.tile([128, CKC, JS], bf16)
            nc.sync.dma_start(fcw_sb[:], fcw_d[:])
            hT = fcp.tile([128, CKC, B], bf16)
            nc.gpsimd.memset(hT[64:, CKC - 1, :], 0.0)
            nc.gpsimd.memset(hT[64:65, CKC - 1, :], 1.0)  # fc bias row
            for bt in range(2):
                hall = fcp.tile([128, CDIM], bf16, tag="hall", bufs=1)
                nc.sync.dma_start(hall[:], h_all_d[bt * 128:(bt + 1) * 128, :])
                for cc in range(CKC):
                    cw = 128 if cc < CKC - 1 else 64
                    pt = psum.tile([128, 128], bf16, tag="tr", bufs=3,
                                   name="ptrh")
                    nc.tensor.transpose(pt[:cw, :], hall[:, ds(cc * 128, cw)],
                                        ident_bf[:])
                    nc.any.tensor_copy(hT[:cw, cc, ts(bt, 128)], pt[:cw, :])
            for h in range(2):
                for cc in range(CKC):
                    nc.tensor.matmul(px[h][:], hT[:, cc, ts(h, 128)],
                                     fcw_sb[:, cc, :],
                                     start=(cc == 0), stop=(cc == CKC - 1))

        # ---------------- recurrent steps ----------------
        def step_body(step, pxs):
            # --- stats: row sums via accum_out, cross-core gather, finalize
            # stat4 cols: [sx_h0, sx_h1, sxx_h0, sxx_h1]
            for h in range(2):
                nc.scalar.activation(xc[:, h, :], pxs[h][:], Act.Identity,
                                     accum_out=stat4[:, h:h + 1])
            for h in range(2):
                nc.scalar.activation(junk[:], xc[:, h, :], Act.Square,
                                     accum_out=stat4[:, 2 + h:3 + h])
            st_in = dram.tile([128, 4], f32, tag="stin", name="st_in")
            st_out = dram.tile([128 * NCORES, 4], f32, tag="stout",
                               addr_space="Shared", name="st_out")
            nc.sync.dma_start(st_in[:], stat4[:])
            nc.gpsimd.collective_compute(
                "AllGather", Alu.bypass, replica_groups=RG,
                ins=[st_in[:].opt()], outs=[st_out[:].opt()])
            stall = work.tile([128, NCORES, 4], f32, tag="stall", name="stall")
            nc.sync.dma_start(stall[:],
                              st_out.rearrange("(r p) f -> p r f", p=128))
            tot = work.tile([128, 4], f32, tag="tot", bufs=3, name="tot")
            nc.vector.tensor_reduce(tot[:], stall.rearrange("p r f -> p f r"),
                                    axis=mybir.AxisListType.X, op=Alu.add)
            # cols of tot: [sx_h0, sx_h1, sxx_h0, sxx_h1]
            totv = tot.rearrange("p (s h) -> p s h", s=2)
            negmu = work.tile([128, 2], f32, tag="fin", bufs=8, name="negmu")
            msq = work.tile([128, 2], f32, tag="fin", bufs=8, name="msq")
            var = work.tile([128, 2], f32, tag="fin", bufs=8, name="var")
            inv2 = work.tile([128, 2], f32, tag="fin", bufs=8, name="inv2")
            ncmu = work.tile([128, 2], f32, tag="fin", bufs=8, name="ncmu")
            nc.vector.tensor_scalar_mul(negmu[:], totv[:, 0, :], -1.0 / N)
            nc.vector.tensor_scalar_mul(var[:], totv[:, 1, :], 1.0 / N)
            nc.vector.tensor_tensor(msq[:], negmu[:], negmu[:], Alu.mult)
            nc.vector.tensor_tensor(var[:], var[:], msq[:], Alu.subtract)
            nc.scalar.activation(var[:], var[:], Act.Sqrt, bias=eps_t[:])
            nc.vector.reciprocal(inv2[:], var[:])
            nc.vector.tensor_tensor(ncmu[:], negmu[:], inv2[:], Alu.mult)

            # --- neuron update per batch-half [128, 512]
            # critical path: t1 -> m1 -> g -> silu -> relu -> transpose.
            # q / aq-for-next-step are off-path (scheduler hoists them).
            for h in range(2):
                # t1 on the DVE so the scalar engine goes straight to Silu —
                # its act-table load hides behind this chain.
                t1 = work.tile([128, JS], f32, tag="t1", name=f"t1{h}")
                nc.vector.tensor_scalar(t1[:], xc[:, h, :],
                                        inv2[:, h:h + 1], ncmu[:, h:h + 1],
                                        Alu.mult, Alu.add)
                m1 = work.tile([128, JS], f32, tag="m1", name=f"m1{h}")
                eng = nc.vector if h == 0 else nc.gpsimd
                eng.tensor_tensor(m1[:], t1[:], lngrep[:], Alu.mult)
                g = work.tile([128, JS], f32, tag="g", name=f"g{h}")
                nc.vector.tensor_tensor(g[:], m1[:], aq_sb[:, h, :], Alu.add)
                sv = work.tile([128, JS], f32, tag="sv", name=f"sv{h}")
                nc.scalar.activation(sv[:], g[:], Act.Silu)
                # spikes = silu(g) * (g>0) == relu(silu(g))
                nc.scalar.activation(s_sb[:, h, :], sv[:], Act.Relu)
                nc.vector.tensor_scalar_min(q_sb[:, h, :], g[:], 0.0)
                nc.vector.scalar_tensor_tensor(aq_sb[:, h, :], q_sb[:, h, :],
                                               LEAK, cbrep[:], Alu.mult,
                                               Alu.add)
        def transpose_spikes(step):
            # transpose spikes to [j, b] bf16, DMA per block pair
            sp_in = dram.tile([JS, B], bf16, tag="spin", name="sp_in")
            for t in range(4):
                for h in range(2):
                    pt = psum.tile([128, 128], bf16, tag="tr", bufs=3,
                                   name="ptr")
                    nc.tensor.transpose(pt[:], s_sb[:, h, ds(t * 128, 128)],
                                        ident_bf[:])
                    if (t * 2 + h) % 2 == 0:
                        nc.scalar.copy(sp_st[:, t, ts(h, 128)], pt[:])
                    else:
                        nc.vector.tensor_copy(sp_st[:, t, ts(h, 128)], pt[:])
                nc.sync.dma_start(sp_in[ds(t * 128, 64), :],
                                  sp_st[0:64, t, :])
                nc.sync.dma_start(sp_in[ds(t * 128 + 64, 64), :],
                                  sp_st[64:128, t, :])
            return sp_in

        def spikes_ag(sp_in):
            ag_out = dram.tile([N, B], bf16, tag="agout", addr_space="Shared",
                               name="ag_out")
            nc.gpsimd.collective_compute(
                "AllGather", Alu.bypass, replica_groups=RG,
                ins=[sp_in[:].opt()], outs=[ag_out[:].opt()])
            return ag_out

        def recurrent_gemm(ag_out):
            # ag row k = r*512 + jl; spT[p, c] holds k(p,c) =
            # (p//16)*512 + (c//16)*256 + (p%16)*16 + (c%16), matching the
            # host-side weight permutation. Progressive loads let the first
            # matmuls start before the whole 512KB lands.
            v = ag_out.rearrange("(p c) b -> p c b", c=KC)
            for (c0, n) in [(0, 1), (1, 1), (2, 2), (4, 4), (8, 8), (16, 16)]:
                nc.sync.dma_start(spT[:, c0:c0 + n, :], v[:, c0:c0 + n, :])
            pxs = [psum.tile([128, JS], f32, tag="gemm", name=f"px{h}")
                   for h in range(2)]
            for h in range(2):
                for c in range(KC):
                    nc.tensor.matmul(pxs[h][:], spT[:, c, ds(h * 128, 128)],
                                     w_sb[:, c, :],
                                     start=(c == 0), stop=(c == KC - 1))
            return pxs

        for step in range(STEPS):
            step_body(step, px)
            sp_ins = transpose_spikes(step)
            if step < STEPS - 1:
                ags = spikes_ag(sp_ins)
                px = recurrent_gemm(ags)

        # ---------------- classifier: partial + AllReduce ----------------
        ps_cls = psum.tile([10, B], f32, tag="cls", bufs=1, name="ps_cls")
        for t in range(4):
            nc.tensor.matmul(ps_cls[:], clsw_sb[:, t, :], sp_st[:, t, :],
                             start=(t == 0), stop=(t == 3))
        cls_loc = work.tile([10, B], f32, tag="clsl", name="cls_loc")
        nc.scalar.copy(cls_loc[:], ps_cls[:])
        cls_in = dram.tile([10, B], f32, bufs=1)
        cls_out = dram.tile([10 * NCORES, B], f32, bufs=1, addr_space="Shared")
        nc.sync.dma_start(cls_in[:], cls_loc[:])
        nc.gpsimd.collective_compute(
            "AllGather", Alu.bypass, replica_groups=RG,
            ins=[cls_in[:].opt()], outs=[cls_out[:].opt()])
        cls_sb = work.tile([10, NCORES, B], f32, tag="clsg", name="cls_sb")
        nc.sync.dma_start(cls_sb[:],
                          cls_out.rearrange("(r p) b -> p r b", p=10))
        acc = work.tile([10, B], f32, tag="clsl", name="acc")
        nc.vector.tensor_tensor(acc[:], cls_sb[:, 0, :], cls_sb[:, 1, :],
                                Alu.add)
        for r in range(2, NCORES):
            nc.vector.tensor_tensor(acc[:], acc[:], cls_sb[:, r, :], Alu.add)
        out_sb = work.tile([10, B], f32, tag="clsl", name="out_sb")
        nc.scalar.activation(out_sb[:], acc[:], Act.Identity,
                             bias=clsb_sb[:])
        nc.sync.dma_start(out_d[:], out_sb[:])

    nc.compile()
    return nc


def _bf16(a):
    """Round fp32 -> bf16 (round-to-nearest-even), keep bf16 dtype via ml_dtypes."""
    import ml_dtypes
    return np.ascontiguousarray(a, np.float32).astype(ml_dtypes.bfloat16)


def _host_prep(inputs):
    """Shard + lay out the full inputs for the 8 cores."""
    x = np.asarray(inputs["x"], np.float32)
    xpad = np.zeros((B, 30, 30), np.float32)
    xpad[:, 1:29, 1:29] = x[:, 0]
    w1t = np.ascontiguousarray(
        np.asarray(inputs["conv1_w"], np.float32).reshape(32, 9).T)

    def _round_f32r(a):
        b = np.ascontiguousarray(a, np.float32).view(np.uint32).astype(np.uint64)
        lsb = (b >> 12) & 1
        out = ((b + 0x7FF + lsb) & 0xFFFFF000).astype(np.uint32)
        return out.view(np.float32)

    w2col = _round_f32r(np.ascontiguousarray(
        np.asarray(inputs["conv2_w"], np.float32).transpose(2, 3, 1, 0)
        .reshape(3, 96, 64)))
    bn1 = np.stack([inputs["bn1_g"], inputs["bn1_b"],
                    inputs["bn1_m"], inputs["bn1_v"]]).astype(np.float32)
    bn2 = np.stack([inputs["bn2_g"], inputs["bn2_b"],
                    inputs["bn2_m"], inputs["bn2_v"]]).astype(np.float32)
    fc_w = np.asarray(inputs["fc_w"], np.float32)
    fc_b = np.asarray(inputs["fc_b"], np.float32)
    rec_w = np.asarray(inputs["rec_w"], np.float32)
    eff_w = rec_w * (np.abs(rec_w) > CONN_THR)
    cls_w = np.asarray(inputs["cls_w"], np.float32)
    clsb = np.ascontiguousarray(
        np.asarray(inputs["cls_b"], np.float32).reshape(10, 1))
    lng = np.asarray(inputs["ln_g"], np.float32)
    lnb = np.asarray(inputs["ln_b"], np.float32)
    thr = np.asarray(inputs["threshold"], np.float32)
    intr = np.asarray(inputs["intrinsic"], np.float32)
    cb = lnb + intr + thr * (LEAK - 1.0)

    in_maps = []
    for r in range(NCORES):
        js = slice(r * JS, (r + 1) * JS)
        # fc weights: [3137 padded to 3200, 512] -> [128, 25, 512], k=c*128+p
        fcp = np.zeros((CKC * 128, JS), np.float32)
        fcp[0:CDIM] = fc_w[js].T
        fcp[CDIM] = fc_b[js]
        fcw = _bf16(fcp.reshape(CKC, 128, JS).transpose(1, 0, 2))
        # recurrent: [4096, 512] -> [128, 32, 512], k = p*32 + c
        wT = _bf16(np.ascontiguousarray(eff_w[js].T).reshape(128, KC, JS))
        clswT = _bf16(np.ascontiguousarray(cls_w[:, js].T)
                      .reshape(4, 128, 10).transpose(1, 0, 2))
        in_maps.append(dict(
            xpad=np.ascontiguousarray(xpad[r * BS:(r + 1) * BS]),
            w1t=w1t, w2col=w2col, bn1=bn1, bn2=bn2,
            fcw=np.ascontiguousarray(fcw),
            wT=np.ascontiguousarray(wT),
            lngr=np.ascontiguousarray(lng[js].reshape(1, JS)),
            cbr=np.ascontiguousarray(cb[js].reshape(1, JS)),
            thrr=np.ascontiguousarray(thr[js].reshape(1, JS)),
            clsw=np.ascontiguousarray(clswT), clsb=clsb,
        ))
    return in_maps


def kernel(**inputs) -> np.ndarray:
    from concourse import bass_utils

    if "nc" not in _PROGRAM_CACHE:
        _PROGRAM_CACHE["nc"] = _build_program()
    nc = _PROGRAM_CACHE["nc"]

    in_maps = _host_prep(inputs)
    res = bass_utils.run_bass_kernel_spmd(
        nc, in_maps, core_ids=list(range(NCORES)))
    _PROGRAM_CACHE["last_results"] = res
    out = res.results[0]["out"]
    return np.ascontiguousarray(out.T.astype(np.float32))
